# revision 38
# baseline (speedup 1.0000x reference)
"""Bass/Tile TRN2 kernel for nn_AttnSCAN: batched attention-like op.

reference (per batch b):
    attn = leaky_relu(context @ query^T, 0.1)            # (c, q)
    attn = attn / (||attn||_2 over q + 1e-8)             # l2norm per c-row
    attn = softmax(9 * attn^T, axis=c)                   # (q, c)
    wcontext = attn @ context                            # (q, d)
    returns (query, wcontext, attn)

Sharding: pure data parallel over the batch dim (128) across 8 cores.

Per-core strategy (16 batches):
  - mm1 contracts over d, so both operands need d on partitions:
    queryT is supplied pre-transposed from the host (query itself is
    only needed as a pass-through output, which the host returns
    directly); contextT is built on-chip with PE transposes.
  - All matmul operands are float32r typed end-to-end (same 4-byte data,
    full-rate PE mode when the moving free dim is >= 256).
  - mm1 produces A in (q, c) layout with free dim 512.
  - leaky_relu in one ACT op (Prelu, alpha=0.1) straight from PSUM.
  - l2norm over q (the partition dim) via ones128.T @ L^2 -> fused
    reduce+broadcast (every output row = sumsq over q); rsqrt computed
    as exp(-0.5*ln(x)) so every ACT op stays in one LUT table set
    (no LoadActFuncSet switches on the critical path).
  - exp fuses the 1/norm scale (scalar_tensor_tensor) and the softmax
    row-sum (activation accum_out).
  - mm2 contracts over c: lhsT = attn^T (8 PE transposes), rhs =
    context in its natural HBM layout.
  - Software pipelining: batch b+1's DMA loads + contextT transposes are
    emitted inside batch b's PE stream, covering the latency of b's
    elementwise softmax chain (ACT/DVE) so the PE never drains.
"""

from contextlib import ExitStack

import numpy as np

import concourse.bass as bass
import concourse.tile as tile
from concourse import bacc, mybir
from concourse.bass_utils import run_bass_kernel_spmd

F32 = mybir.dt.float32
F32R = mybir.dt.float32r
AF = mybir.ActivationFunctionType
OP = mybir.AluOpType

N_CORES = 8
B_TOTAL = 128
NQ = 128
NCTX = 1024
D = 1024

NEG_SLOPE = 0.1
SMOOTH = 9.0


_ACT_SET = "natural_log_exp_and_others"
_patched_tables = False


def _patch_act_tables():
    """Make every ACT function this kernel uses resolve to one LUT set so
    bacc emits a single LoadActFuncSet instead of per-batch switches. Set
    indices (= act_func_set_id) are preserved; we only hide our functions
    from the other sets so the chooser can't oscillate."""
    global _patched_tables
    if _patched_tables:
        return
    _patched_tables = True
    import concourse.hw_specs as hw_specs

    mine = {AF.Exp, AF.Ln, AF.Square, AF.Copy, AF.Identity, AF.Relu}
    orig = hw_specs.get_activation_tables

    def patched(module_arch):
        tables = dict(orig(module_arch))
        assert _ACT_SET in tables and mine <= tables[_ACT_SET]
        return {
            name: (funcs if name == _ACT_SET else funcs - mine)
            for name, funcs in tables.items()
        }

    hw_specs.get_activation_tables = patched
    import concourse.bacc as bacc_mod

    for mod in (bacc_mod,):
        if getattr(mod, "get_activation_tables", None) is orig:
            mod.get_activation_tables = patched


def build_nc(nb: int):
    """Build the per-core Bass module processing `nb` batches."""
    _patch_act_tables()
    nc = bacc.Bacc("TRN2", target_bir_lowering=False, debug=False)

    qT_d = nc.dram_tensor("queryT", (nb, 128, 8 * NQ), F32R, kind="ExternalInput")
    c_d = nc.dram_tensor("context", (nb, NCTX, D), F32R, kind="ExternalInput")
    ident_d = nc.dram_tensor("ident", (128, 128), F32R, kind="ExternalInput")
    ones_d = nc.dram_tensor("ones", (128, 128), F32R, kind="ExternalInput")
    attn_d = nc.dram_tensor("attn", (nb, NQ, NCTX), F32R, kind="ExternalOutput")
    w_d = nc.dram_tensor("wcontext", (nb, NQ, D), F32, kind="ExternalOutput")

    with tile.TileContext(nc) as tc, ExitStack() as ctx:
        Body(ctx, tc, nb, qT_d, c_d, ident_d, ones_d, attn_d, w_d).run()
    nc.compile()
    return nc


class Body:
    def __init__(self, ctx, tc, nb, qT_d, c_d, ident_d, ones_d, attn_d, w_d):
        self.tc = tc
        self.nc = tc.nc
        self.nb = nb
        self.qT_ap = qT_d.ap()
        self.c_ap = c_d.ap()
        self.attn_ap = attn_d.ap()
        self.w_ap = w_d.ap()
        nc = self.nc

        self.const = ctx.enter_context(tc.tile_pool(name="const", bufs=1))
        self.ctx_pool = ctx.enter_context(tc.tile_pool(name="ctx", bufs=3))
        self.ct_pool = ctx.enter_context(tc.tile_pool(name="ct", bufs=2))
        self.qt_pool = ctx.enter_context(tc.tile_pool(name="qt", bufs=3))
        self.w1 = ctx.enter_context(tc.tile_pool(name="w1", bufs=1))
        self.pt_pool = ctx.enter_context(
            tc.tile_pool(name="pt", bufs=2, space=bass.MemorySpace.PSUM)
        )
        self.pa_pool = ctx.enter_context(
            tc.tile_pool(name="pa", bufs=2, space=bass.MemorySpace.PSUM)
        )
        self.pw_pool = ctx.enter_context(
            tc.tile_pool(name="pw", bufs=2, space=bass.MemorySpace.PSUM)
        )

        self.ident_sb = self.const.tile([128, 128], F32R, tag="ident")
        nc.sync.dma_start(self.ident_sb[:], ident_d.ap())
        self.ones_sb = self.const.tile([128, 128], F32R, tag="ones")
        self._ones_d = ones_d

        # per-batch live tiles, keyed b % depth
        self.ctx_sb = {}
        self.qT_sb = {}
        self.cT_sb = {}

    # ---- pipeline stages ----

    def load(self, b):
        nc = self.nc
        ctx_sb = self.ctx_pool.tile([128, 8 * D], F32R, tag="ctx")
        nslc = 4
        w = D // nslc
        for dh in range(nslc):
            nc.sync.dma_start(
                ctx_sb[:].rearrange("p (t d) -> p t d", d=D)[
                    :, :, dh * w : (dh + 1) * w
                ],
                self.c_ap[b].rearrange("(t p) d -> p t d", p=128)[
                    :, :, dh * w : (dh + 1) * w
                ],
            )
        qT_sb = self.qt_pool.tile([128, 8 * NQ], F32R, tag="qT")
        nc.sync.dma_start(qT_sb[:], self.qT_ap[b])
        self.ctx_sb[b] = ctx_sb
        self.qT_sb[b] = qT_sb
        self.cT_sb[b] = self.ct_pool.tile(
            [128, 8 * NCTX], F32R, tag="cT", name=f"cT{b}"
        )

    def transpose_ctx(self, b, j_range):
        """PE-transpose context d-chunks j into cT (staged via PSUM)."""
        nc = self.nc
        ctx_sb, cT_sb = self.ctx_sb[b], self.cT_sb[b]
        for j in j_range:  # d-chunk
            pt = self.pt_pool.tile([128, 1024], F32R, tag="pt")
            for i in range(8):  # c-chunk
                nc.tensor.transpose(
                    pt[:, i * 128 : (i + 1) * 128],
                    ctx_sb[:, i * D + j * 128 : i * D + (j + 1) * 128],
                    self.ident_sb[:],
                )
            dst = cT_sb[:, j * NCTX : (j + 1) * NCTX]
            if j % 2 == 0:
                nc.vector.tensor_copy(dst, pt[:])
            else:
                nc.scalar.copy(dst, pt[:])

    def mm1(self, b):
        nc = self.nc
        qT_sb, cT_sb = self.qT_sb[b], self.cT_sb[b]
        pa = [
            self.pa_pool.tile([128, 512], F32, tag="pa", name=f"pa{b}_{n}")
            for n in range(2)
        ]
        for n in range(2):
            for j in range(8):
                nc.tensor.matmul(
                    pa[n][:],
                    qT_sb[:, j * NQ : (j + 1) * NQ],
                    cT_sb[:, j * NCTX + n * 512 : j * NCTX + (n + 1) * 512],
                    start=(j == 0),
                    stop=(j == 7),
                )
        return pa

    def leaky_sq(self, b, pa, h):
        """Half h: lk = max(0.1*pa, pa); lsq = lk^2; sumsq matmul."""
        nc = self.nc
        sl = slice(h * 512, (h + 1) * 512)
        if h == 0:
            self._lk = self.w1.tile([128, 1024], F32, tag="lk")
            self._lsq = self.w1.tile([128, 1024], F32R, tag="lsq")
        lk, lsq = self._lk, self._lsq
        nc.scalar.activation(lk[:, sl], pa[h][:], AF.Prelu, alpha=NEG_SLOPE)
        nc.vector.tensor_tensor(
            lsq[:, sl], lk[:, sl], lk[:, sl], op=OP.mult
        )
        ps = self.pa_pool.tile([128, 512], F32, tag="pa", name=f"ps2_{b}_{h}")
        nc.tensor.matmul(
            ps[:], self.ones_sb[:], lsq[:, sl], start=True, stop=True
        )
        return lk, ps

    def softmax(self, b, lk, ps2):
        """rsqrt via exp(-0.5*ln(x)) keeps every ACT op in one LUT set
        (natural_log_exp_and_others) -> no LoadActFuncSet switches."""
        nc = self.nc
        lnt = self.w1.tile([128, 1024], F32, tag="lnt")
        ru = lnt
        x9 = self.w1.tile([128, 1024], F32, tag="x9")
        ex = self.w1.tile([128, 1024], F32, tag="ex")
        for h in range(2):
            sl = slice(h * 512, (h + 1) * 512)
            nc.scalar.activation(lnt[:, sl], ps2[h][:], AF.Ln)
            nc.scalar.activation(ru[:, sl], lnt[:, sl], AF.Exp, scale=-0.5)
            nc.vector.scalar_tensor_tensor(
                x9[:, sl], lk[:, sl], SMOOTH, ru[:, sl], op0=OP.mult, op1=OP.mult
            )
        s_tot = self.w1.tile([128, 1], F32, tag="s_tot")
        nc.scalar.activation(ex[:], x9[:], AF.Exp, accum_out=s_tot[:])
        rs = self.w1.tile([128, 1], F32, tag="rs")
        nc.vector.reciprocal(rs[:], s_tot[:])
        attn_sb = self.w1.tile([128, 1024], F32R, tag="attn")
        for h in range(2):
            sl = slice(h * 512, (h + 1) * 512)
            nc.vector.tensor_scalar_mul(attn_sb[:, sl], ex[:, sl], rs[:])
            nc.sync.dma_start(self.attn_ap[b][:, sl], attn_sb[:, sl])
        return attn_sb

    def mm2(self, b, attn_sb):
        nc = self.nc
        et_sb = self.w1.tile([128, 1024], F32R, tag="et")
        pt = self.pt_pool.tile([128, 1024], F32R, tag="pt")
        for i in range(8):
            nc.tensor.transpose(
                pt[:, i * 128 : (i + 1) * 128],
                attn_sb[:, i * 128 : (i + 1) * 128],
                self.ident_sb[:],
            )
        nc.vector.tensor_copy(et_sb[:], pt[:])

        ctx_sb = self.ctx_sb[b]
        pw = [
            self.pw_pool.tile([128, 512], F32, tag="pw", name=f"pw{b}_{n}")
            for n in range(2)
        ]
        w_sb = self.w1.tile([128, 1024], F32, tag="w_sb")
        for n in range(2):
            for i in range(8):
                nc.tensor.matmul(
                    pw[n][:],
                    et_sb[:, i * 128 : (i + 1) * 128],
                    ctx_sb[:, i * D + n * 512 : i * D + (n + 1) * 512],
                    start=(i == 0),
                    stop=(i == 7),
                )
            nc.scalar.copy(w_sb[:, n * 512 : (n + 1) * 512], pw[n][:])
            nc.sync.dma_start(
                self.w_ap[b][:, n * 512 : (n + 1) * 512],
                w_sb[:, n * 512 : (n + 1) * 512],
            )
        # inputs for batch b fully consumed
        del self.ctx_sb[b], self.qT_sb[b], self.cT_sb[b]

    def run(self):
        nb = self.nb
        # prologue: stage batch 0 fully, prefetch batch 1
        self.load(0)
        self.nc.sync.dma_start(self.ones_sb[:], self._ones_d.ap())
        if nb > 1:
            self.load(1)
        self.transpose_ctx(0, range(8))
        for b in range(nb):
            pa = self.mm1(b)
            lk, ps0 = self.leaky_sq(b, pa, 0)
            if b + 2 < nb:
                self.load(b + 2)
            if b + 1 < nb:
                self.transpose_ctx(b + 1, range(0, 2))
            _, ps1 = self.leaky_sq(b, pa, 1)
            if b + 1 < nb:
                self.transpose_ctx(b + 1, range(2, 5))
            attn_sb = self.softmax(b, lk, [ps0, ps1])
            if b + 1 < nb:
                self.transpose_ctx(b + 1, range(5, 8))
            if b == nb - 2:
                # Tail: start the final batch's mm1 + softmax before mm2(b)
                # so its ACT/DVE chain overlaps both mm2s on the PE.
                bl = nb - 1
                pa_l = self.mm1(bl)
                lk_l, ps0_l = self.leaky_sq(bl, pa_l, 0)
                _, ps1_l = self.leaky_sq(bl, pa_l, 1)
                attn_l = self.softmax(bl, lk_l, [ps0_l, ps1_l])
                self.mm2(b, attn_sb)
                self.mm2(bl, attn_l)
                break
            self.mm2(b, attn_sb)


_NC_CACHE = {}


def get_nc(nb: int):
    if nb not in _NC_CACHE:
        _NC_CACHE[nb] = build_nc(nb)
    return _NC_CACHE[nb]


def make_in_maps(query: np.ndarray, context: np.ndarray):
    """Shard full inputs into per-core input maps."""
    n = query.shape[0]
    per = n // N_CORES
    # qT[b, d, q] -> [b, p, j, q] with d = j*128 + p: partition-contiguous DMA
    qT = query.transpose(0, 2, 1).reshape(n, 8, 128, NQ)
    qT = np.ascontiguousarray(qT.transpose(0, 2, 1, 3)).reshape(n, 128, 8 * NQ)
    ident = np.eye(128, dtype=np.float32)
    ones = np.ones((128, 128), dtype=np.float32)
    in_maps = []
    for c in range(N_CORES):
        sl = slice(c * per, (c + 1) * per)
        in_maps.append(
            {
                "queryT": qT[sl],
                "context": np.ascontiguousarray(context[sl]),
                "ident": ident,
                "ones": ones,
            }
        )
    return in_maps


def kernel(query: np.ndarray, context: np.ndarray):
    query = np.asarray(query, dtype=np.float32)
    context = np.asarray(context, dtype=np.float32)
    assert query.shape == (B_TOTAL, NQ, D) and context.shape == (B_TOTAL, NCTX, D)
    per = B_TOTAL // N_CORES

    nc = get_nc(per)
    in_maps = make_in_maps(query, context)
    res = run_bass_kernel_spmd(nc, in_maps, list(range(N_CORES)))
    attn = np.concatenate([r["attn"] for r in res.results], axis=0)
    wctx = np.concatenate([r["wcontext"] for r in res.results], axis=0)
    return (query, wctx, attn)


# revision 45
# speedup vs baseline: 1.0026x; 1.0026x over previous
"""Bass/Tile TRN2 kernel for nn_AttnSCAN: batched attention-like op.

reference (per batch b):
    attn = leaky_relu(context @ query^T, 0.1)            # (c, q)
    attn = attn / (||attn||_2 over q + 1e-8)             # l2norm per c-row
    attn = softmax(9 * attn^T, axis=c)                   # (q, c)
    wcontext = attn @ context                            # (q, d)
    returns (query, wcontext, attn)

Sharding: pure data parallel over the batch dim (128) across 8 cores.

Per-core strategy (16 batches):
  - mm1 contracts over d, so both operands need d on partitions:
    queryT is supplied pre-transposed from the host (query itself is
    only needed as a pass-through output, which the host returns
    directly); contextT is built on-chip with PE transposes.
  - All matmul operands are float32r typed end-to-end (same 4-byte data,
    full-rate PE mode when the moving free dim is >= 256).
  - mm1 produces A in (q, c) layout with free dim 512.
  - leaky_relu in one ACT op (Prelu, alpha=0.1) straight from PSUM.
  - l2norm over q (the partition dim) via ones128.T @ L^2 -> fused
    reduce+broadcast (every output row = sumsq over q); rsqrt computed
    as exp(-0.5*ln(x)) so every ACT op stays in one LUT table set
    (no LoadActFuncSet switches on the critical path).
  - exp fuses the 1/norm scale (scalar_tensor_tensor) and the softmax
    row-sum (activation accum_out).
  - mm2 contracts over c: lhsT = attn^T (8 PE transposes), rhs =
    context in its natural HBM layout.
  - Software pipelining: batch b+1's DMA loads + contextT transposes are
    emitted inside batch b's PE stream, covering the latency of b's
    elementwise softmax chain (ACT/DVE) so the PE never drains.
"""

from contextlib import ExitStack

import numpy as np

import concourse.bass as bass
import concourse.tile as tile
from concourse import bacc, mybir
from concourse.bass_utils import run_bass_kernel_spmd

F32 = mybir.dt.float32
F32R = mybir.dt.float32r
AF = mybir.ActivationFunctionType
OP = mybir.AluOpType

N_CORES = 8
B_TOTAL = 128
NQ = 128
NCTX = 1024
D = 1024

NEG_SLOPE = 0.1
SMOOTH = 9.0


_ACT_SET = "natural_log_exp_and_others"
_patched_tables = False


def _patch_act_tables():
    """Make every ACT function this kernel uses resolve to one LUT set so
    bacc emits a single LoadActFuncSet instead of per-batch switches. Set
    indices (= act_func_set_id) are preserved; we only hide our functions
    from the other sets so the chooser can't oscillate."""
    global _patched_tables
    if _patched_tables:
        return
    _patched_tables = True
    import concourse.hw_specs as hw_specs

    mine = {AF.Exp, AF.Ln, AF.Square, AF.Copy, AF.Identity, AF.Relu}
    orig = hw_specs.get_activation_tables

    def patched(module_arch):
        tables = dict(orig(module_arch))
        assert _ACT_SET in tables and mine <= tables[_ACT_SET]
        return {
            name: (funcs if name == _ACT_SET else funcs - mine)
            for name, funcs in tables.items()
        }

    hw_specs.get_activation_tables = patched
    import concourse.bacc as bacc_mod

    for mod in (bacc_mod,):
        if getattr(mod, "get_activation_tables", None) is orig:
            mod.get_activation_tables = patched


def build_nc(nb: int):
    """Build the per-core Bass module processing `nb` batches."""
    _patch_act_tables()
    nc = bacc.Bacc("TRN2", target_bir_lowering=False, debug=False)

    qT_d = nc.dram_tensor("queryT", (nb, 128, 8 * NQ), F32R, kind="ExternalInput")
    c_d = nc.dram_tensor("context", (nb, NCTX, D), F32R, kind="ExternalInput")
    ident_d = nc.dram_tensor("ident", (128, 128), F32R, kind="ExternalInput")
    ones_d = nc.dram_tensor("ones", (128, 128), F32R, kind="ExternalInput")
    attn_d = nc.dram_tensor("attn", (nb, NQ, NCTX), F32R, kind="ExternalOutput")
    w_d = nc.dram_tensor("wcontext", (nb, NQ, D), F32, kind="ExternalOutput")

    with tile.TileContext(nc) as tc, ExitStack() as ctx:
        Body(ctx, tc, nb, qT_d, c_d, ident_d, ones_d, attn_d, w_d).run()
    nc.compile()
    return nc


class Body:
    def __init__(self, ctx, tc, nb, qT_d, c_d, ident_d, ones_d, attn_d, w_d):
        self.tc = tc
        self.nc = tc.nc
        self.nb = nb
        self.qT_ap = qT_d.ap()
        self.c_ap = c_d.ap()
        self.attn_ap = attn_d.ap()
        self.w_ap = w_d.ap()
        nc = self.nc

        self.const = ctx.enter_context(tc.tile_pool(name="const", bufs=1))
        self.ctx_pool = ctx.enter_context(tc.tile_pool(name="ctx", bufs=3))
        self.ct_pool = ctx.enter_context(tc.tile_pool(name="ct", bufs=2))
        self.qt_pool = ctx.enter_context(tc.tile_pool(name="qt", bufs=3))
        self.w1 = ctx.enter_context(tc.tile_pool(name="w1", bufs=1))
        self.pt_pool = ctx.enter_context(
            tc.tile_pool(name="pt", bufs=2, space=bass.MemorySpace.PSUM)
        )
        self.pa_pool = ctx.enter_context(
            tc.tile_pool(name="pa", bufs=2, space=bass.MemorySpace.PSUM)
        )
        self.pw_pool = ctx.enter_context(
            tc.tile_pool(name="pw", bufs=2, space=bass.MemorySpace.PSUM)
        )

        self.ident_sb = self.const.tile([128, 128], F32R, tag="ident")
        # SWDGE path: independent issue queue, so the first HWDGE slot
        # goes to context slice 0 instead of this tiny constant
        nc.gpsimd.dma_start(self.ident_sb[:], ident_d.ap())
        self.ones_sb = self.const.tile([128, 128], F32R, tag="ones")
        self._ones_d = ones_d

        # per-batch live tiles, keyed b % depth
        self.ctx_sb = {}
        self.qT_sb = {}
        self.cT_sb = {}

    # ---- pipeline stages ----

    def load(self, b):
        nc = self.nc
        ctx_sb = self.ctx_pool.tile([128, 8 * D], F32R, tag="ctx")
        nslc = 4
        w = D // nslc
        for dh in range(nslc):
            nc.sync.dma_start(
                ctx_sb[:].rearrange("p (t d) -> p t d", d=D)[
                    :, :, dh * w : (dh + 1) * w
                ],
                self.c_ap[b].rearrange("(t p) d -> p t d", p=128)[
                    :, :, dh * w : (dh + 1) * w
                ],
            )
        qT_sb = self.qt_pool.tile([128, 8 * NQ], F32R, tag="qT")
        nc.sync.dma_start(qT_sb[:], self.qT_ap[b])
        self.ctx_sb[b] = ctx_sb
        self.qT_sb[b] = qT_sb
        self.cT_sb[b] = self.ct_pool.tile(
            [128, 8 * NCTX], F32R, tag="cT", name=f"cT{b}"
        )

    def transpose_ctx(self, b, j_range):
        """PE-transpose context d-chunks j into cT (staged via PSUM)."""
        nc = self.nc
        ctx_sb, cT_sb = self.ctx_sb[b], self.cT_sb[b]
        for j in j_range:  # d-chunk
            pt = self.pt_pool.tile([128, 1024], F32R, tag="pt")
            for i in range(8):  # c-chunk
                nc.tensor.transpose(
                    pt[:, i * 128 : (i + 1) * 128],
                    ctx_sb[:, i * D + j * 128 : i * D + (j + 1) * 128],
                    self.ident_sb[:],
                )
            dst = cT_sb[:, j * NCTX : (j + 1) * NCTX]
            if j % 2 == 0:
                nc.vector.tensor_copy(dst, pt[:])
            else:
                nc.scalar.copy(dst, pt[:])

    def mm1(self, b):
        nc = self.nc
        qT_sb, cT_sb = self.qT_sb[b], self.cT_sb[b]
        pa = [
            self.pa_pool.tile([128, 512], F32, tag="pa", name=f"pa{b}_{n}")
            for n in range(2)
        ]
        for n in range(2):
            for j in range(8):
                nc.tensor.matmul(
                    pa[n][:],
                    qT_sb[:, j * NQ : (j + 1) * NQ],
                    cT_sb[:, j * NCTX + n * 512 : j * NCTX + (n + 1) * 512],
                    start=(j == 0),
                    stop=(j == 7),
                )
        return pa

    def leaky_sq(self, b, pa, h):
        """Half h: lk = max(0.1*pa, pa); lsq = lk^2; sumsq matmul."""
        nc = self.nc
        sl = slice(h * 512, (h + 1) * 512)
        if h == 0:
            self._lk = self.w1.tile([128, 1024], F32, tag="lk")
            self._lsq = self.w1.tile([128, 1024], F32R, tag="lsq")
        lk, lsq = self._lk, self._lsq
        nc.scalar.activation(lk[:, sl], pa[h][:], AF.Prelu, alpha=NEG_SLOPE)
        nc.vector.tensor_tensor(
            lsq[:, sl], lk[:, sl], lk[:, sl], op=OP.mult
        )
        ps = self.pa_pool.tile([128, 512], F32, tag="pa", name=f"ps2_{b}_{h}")
        nc.tensor.matmul(
            ps[:], self.ones_sb[:], lsq[:, sl], start=True, stop=True
        )
        return lk, ps

    def softmax(self, b, lk, ps2):
        """rsqrt via exp(-0.5*ln(x)) keeps every ACT op in one LUT set
        (natural_log_exp_and_others) -> no LoadActFuncSet switches."""
        nc = self.nc
        lnt = self.w1.tile([128, 1024], F32, tag="lnt")
        ru = lnt
        x9 = self.w1.tile([128, 1024], F32, tag="x9")
        ex = self.w1.tile([128, 1024], F32, tag="ex")
        for h in range(2):
            sl = slice(h * 512, (h + 1) * 512)
            nc.scalar.activation(lnt[:, sl], ps2[h][:], AF.Ln)
            nc.scalar.activation(ru[:, sl], lnt[:, sl], AF.Exp, scale=-0.5)
            nc.vector.scalar_tensor_tensor(
                x9[:, sl], lk[:, sl], SMOOTH, ru[:, sl], op0=OP.mult, op1=OP.mult
            )
        s_tot = self.w1.tile([128, 1], F32, tag="s_tot")
        nc.scalar.activation(ex[:], x9[:], AF.Exp, accum_out=s_tot[:])
        rs = self.w1.tile([128, 1], F32, tag="rs")
        nc.vector.reciprocal(rs[:], s_tot[:])
        attn_sb = self.w1.tile([128, 1024], F32R, tag="attn")
        for h in range(2):
            sl = slice(h * 512, (h + 1) * 512)
            nc.vector.tensor_scalar_mul(attn_sb[:, sl], ex[:, sl], rs[:])
            nc.sync.dma_start(self.attn_ap[b][:, sl], attn_sb[:, sl])
        return attn_sb

    def mm2(self, b, attn_sb):
        nc = self.nc
        et_sb = self.w1.tile([128, 1024], F32R, tag="et")
        pt = self.pt_pool.tile([128, 1024], F32R, tag="pt")
        for i in range(8):
            nc.tensor.transpose(
                pt[:, i * 128 : (i + 1) * 128],
                attn_sb[:, i * 128 : (i + 1) * 128],
                self.ident_sb[:],
            )
        nc.vector.tensor_copy(et_sb[:], pt[:])

        ctx_sb = self.ctx_sb[b]
        pw = [
            self.pw_pool.tile([128, 512], F32, tag="pw", name=f"pw{b}_{n}")
            for n in range(2)
        ]
        w_sb = self.w1.tile([128, 1024], F32, tag="w_sb")
        for n in range(2):
            for i in range(8):
                nc.tensor.matmul(
                    pw[n][:],
                    et_sb[:, i * 128 : (i + 1) * 128],
                    ctx_sb[:, i * D + n * 512 : i * D + (n + 1) * 512],
                    start=(i == 0),
                    stop=(i == 7),
                )
            nc.scalar.copy(w_sb[:, n * 512 : (n + 1) * 512], pw[n][:])
            nc.sync.dma_start(
                self.w_ap[b][:, n * 512 : (n + 1) * 512],
                w_sb[:, n * 512 : (n + 1) * 512],
            )
        # inputs for batch b fully consumed
        del self.ctx_sb[b], self.qT_sb[b], self.cT_sb[b]

    def run(self):
        nb = self.nb
        # prologue: stage batch 0 fully, prefetch batch 1
        self.load(0)
        self.nc.sync.dma_start(self.ones_sb[:], self._ones_d.ap())
        if nb > 1:
            self.load(1)
        self.transpose_ctx(0, range(8))
        for b in range(nb):
            pa = self.mm1(b)
            lk, ps0 = self.leaky_sq(b, pa, 0)
            if b + 2 < nb:
                self.load(b + 2)
            if b + 1 < nb:
                self.transpose_ctx(b + 1, range(0, 2))
            _, ps1 = self.leaky_sq(b, pa, 1)
            if b + 1 < nb:
                self.transpose_ctx(b + 1, range(2, 5))
            attn_sb = self.softmax(b, lk, [ps0, ps1])
            if b + 1 < nb:
                self.transpose_ctx(b + 1, range(5, 8))
            if b == nb - 2:
                # Tail: start the final batch's mm1 + softmax before mm2(b)
                # so its ACT/DVE chain overlaps both mm2s on the PE.
                bl = nb - 1
                pa_l = self.mm1(bl)
                lk_l, ps0_l = self.leaky_sq(bl, pa_l, 0)
                _, ps1_l = self.leaky_sq(bl, pa_l, 1)
                attn_l = self.softmax(bl, lk_l, [ps0_l, ps1_l])
                self.mm2(b, attn_sb)
                self.mm2(bl, attn_l)
                break
            self.mm2(b, attn_sb)


_NC_CACHE = {}


def get_nc(nb: int):
    if nb not in _NC_CACHE:
        _NC_CACHE[nb] = build_nc(nb)
    return _NC_CACHE[nb]


def make_in_maps(query: np.ndarray, context: np.ndarray):
    """Shard full inputs into per-core input maps."""
    n = query.shape[0]
    per = n // N_CORES
    # qT[b, d, q] -> [b, p, j, q] with d = j*128 + p: partition-contiguous DMA
    qT = query.transpose(0, 2, 1).reshape(n, 8, 128, NQ)
    qT = np.ascontiguousarray(qT.transpose(0, 2, 1, 3)).reshape(n, 128, 8 * NQ)
    ident = np.eye(128, dtype=np.float32)
    ones = np.ones((128, 128), dtype=np.float32)
    in_maps = []
    for c in range(N_CORES):
        sl = slice(c * per, (c + 1) * per)
        in_maps.append(
            {
                "queryT": qT[sl],
                "context": np.ascontiguousarray(context[sl]),
                "ident": ident,
                "ones": ones,
            }
        )
    return in_maps


def kernel(query: np.ndarray, context: np.ndarray):
    query = np.asarray(query, dtype=np.float32)
    context = np.asarray(context, dtype=np.float32)
    assert query.shape == (B_TOTAL, NQ, D) and context.shape == (B_TOTAL, NCTX, D)
    per = B_TOTAL // N_CORES

    nc = get_nc(per)
    in_maps = make_in_maps(query, context)
    res = run_bass_kernel_spmd(nc, in_maps, list(range(N_CORES)))
    attn = np.concatenate([r["attn"] for r in res.results], axis=0)
    wctx = np.concatenate([r["wcontext"] for r in res.results], axis=0)
    return (query, wctx, attn)


# revision 49
# speedup vs baseline: 1.0031x; 1.0005x over previous
"""Bass/Tile TRN2 kernel for nn_AttnSCAN: batched attention-like op.

reference (per batch b):
    attn = leaky_relu(context @ query^T, 0.1)            # (c, q)
    attn = attn / (||attn||_2 over q + 1e-8)             # l2norm per c-row
    attn = softmax(9 * attn^T, axis=c)                   # (q, c)
    wcontext = attn @ context                            # (q, d)
    returns (query, wcontext, attn)

Sharding: pure data parallel over the batch dim (128) across 8 cores.

Per-core strategy (16 batches):
  - mm1 contracts over d, so both operands need d on partitions:
    queryT is supplied pre-transposed from the host (query itself is
    only needed as a pass-through output, which the host returns
    directly); contextT is built on-chip with PE transposes.
  - All matmul operands are float32r typed end-to-end (same 4-byte data,
    full-rate PE mode when the moving free dim is >= 256).
  - mm1 produces A in (q, c) layout with free dim 512.
  - leaky_relu in one ACT op (Prelu, alpha=0.1) straight from PSUM.
  - l2norm over q (the partition dim) via ones128.T @ L^2 -> fused
    reduce+broadcast (every output row = sumsq over q); rsqrt computed
    as exp(-0.5*ln(x)) so every ACT op stays in one LUT table set
    (no LoadActFuncSet switches on the critical path).
  - exp fuses the 1/norm scale (scalar_tensor_tensor) and the softmax
    row-sum (activation accum_out).
  - mm2 contracts over c: lhsT = attn^T (8 PE transposes), rhs =
    context in its natural HBM layout.
  - Software pipelining: batch b+1's DMA loads + contextT transposes are
    emitted inside batch b's PE stream, covering the latency of b's
    elementwise softmax chain (ACT/DVE) so the PE never drains.
"""

from contextlib import ExitStack

import numpy as np

import concourse.bass as bass
import concourse.tile as tile
from concourse import bacc, mybir
from concourse.bass_utils import run_bass_kernel_spmd

F32 = mybir.dt.float32
F32R = mybir.dt.float32r
AF = mybir.ActivationFunctionType
OP = mybir.AluOpType

N_CORES = 8
B_TOTAL = 128
NQ = 128
NCTX = 1024
D = 1024

NEG_SLOPE = 0.1
SMOOTH = 9.0


_ACT_SET = "natural_log_exp_and_others"
_patched_tables = False


def _patch_act_tables():
    """Make every ACT function this kernel uses resolve to one LUT set so
    bacc emits a single LoadActFuncSet instead of per-batch switches. Set
    indices (= act_func_set_id) are preserved; we only hide our functions
    from the other sets so the chooser can't oscillate."""
    global _patched_tables
    if _patched_tables:
        return
    _patched_tables = True
    import concourse.hw_specs as hw_specs

    mine = {AF.Exp, AF.Ln, AF.Square, AF.Copy, AF.Identity, AF.Relu}
    orig = hw_specs.get_activation_tables

    def patched(module_arch):
        tables = dict(orig(module_arch))
        assert _ACT_SET in tables and mine <= tables[_ACT_SET]
        return {
            name: (funcs if name == _ACT_SET else funcs - mine)
            for name, funcs in tables.items()
        }

    hw_specs.get_activation_tables = patched
    import concourse.bacc as bacc_mod

    for mod in (bacc_mod,):
        if getattr(mod, "get_activation_tables", None) is orig:
            mod.get_activation_tables = patched


def build_nc(nb: int):
    """Build the per-core Bass module processing `nb` batches."""
    _patch_act_tables()
    nc = bacc.Bacc("TRN2", target_bir_lowering=False, debug=False)

    qT_d = nc.dram_tensor("queryT", (nb, 128, 8 * NQ), F32R, kind="ExternalInput")
    c_d = nc.dram_tensor("context", (nb, NCTX, D), F32R, kind="ExternalInput")
    ident_d = nc.dram_tensor("ident", (128, 128), F32R, kind="ExternalInput")
    ones_d = nc.dram_tensor("ones", (128, 128), F32R, kind="ExternalInput")
    attn_d = nc.dram_tensor("attn", (nb, NQ, NCTX), F32R, kind="ExternalOutput")
    w_d = nc.dram_tensor("wcontext", (nb, NQ, D), F32, kind="ExternalOutput")

    with tile.TileContext(nc) as tc, ExitStack() as ctx:
        Body(ctx, tc, nb, qT_d, c_d, ident_d, ones_d, attn_d, w_d).run()
    nc.compile()
    return nc


class Body:
    def __init__(self, ctx, tc, nb, qT_d, c_d, ident_d, ones_d, attn_d, w_d):
        self.tc = tc
        self.nc = tc.nc
        self.nb = nb
        self.qT_ap = qT_d.ap()
        self.c_ap = c_d.ap()
        self.attn_ap = attn_d.ap()
        self.w_ap = w_d.ap()
        nc = self.nc

        self.const = ctx.enter_context(tc.tile_pool(name="const", bufs=1))
        self.ctx_pool = ctx.enter_context(tc.tile_pool(name="ctx", bufs=3))
        self.ct_pool = ctx.enter_context(tc.tile_pool(name="ct", bufs=2))
        self.qt_pool = ctx.enter_context(tc.tile_pool(name="qt", bufs=3))
        self.w1 = ctx.enter_context(tc.tile_pool(name="w1", bufs=1))
        self.pt_pool = ctx.enter_context(
            tc.tile_pool(name="pt", bufs=2, space=bass.MemorySpace.PSUM)
        )
        self.pa_pool = ctx.enter_context(
            tc.tile_pool(name="pa", bufs=2, space=bass.MemorySpace.PSUM)
        )
        self.pw_pool = ctx.enter_context(
            tc.tile_pool(name="pw", bufs=2, space=bass.MemorySpace.PSUM)
        )

        self.ident_sb = self.const.tile([128, 128], F32R, tag="ident")
        # SWDGE path: independent issue queue, so the first HWDGE slot
        # goes to context slice 0 instead of this tiny constant
        nc.gpsimd.dma_start(self.ident_sb[:], ident_d.ap())
        self.ones_sb = self.const.tile([128, 128], F32R, tag="ones")
        self._ones_d = ones_d

        # per-batch live tiles, keyed b % depth
        self.ctx_sb = {}
        self.qT_sb = {}
        self.cT_sb = {}

    # ---- pipeline stages ----

    def load(self, b):
        nc = self.nc
        ctx_sb = self.ctx_pool.tile([128, 8 * D], F32R, tag="ctx")
        nslc = 4
        w = D // nslc
        for dh in range(nslc):
            nc.sync.dma_start(
                ctx_sb[:].rearrange("p (t d) -> p t d", d=D)[
                    :, :, dh * w : (dh + 1) * w
                ],
                self.c_ap[b].rearrange("(t p) d -> p t d", p=128)[
                    :, :, dh * w : (dh + 1) * w
                ],
            )
        qT_sb = self.qt_pool.tile([128, 8 * NQ], F32R, tag="qT")
        nc.sync.dma_start(qT_sb[:], self.qT_ap[b])
        self.ctx_sb[b] = ctx_sb
        self.qT_sb[b] = qT_sb
        self.cT_sb[b] = self.ct_pool.tile(
            [128, 8 * NCTX], F32R, tag="cT", name=f"cT{b}"
        )

    def transpose_ctx(self, b, j_range):
        """PE-transpose context d-chunks j into cT (staged via PSUM)."""
        nc = self.nc
        ctx_sb, cT_sb = self.ctx_sb[b], self.cT_sb[b]
        for j in j_range:  # d-chunk
            pt = self.pt_pool.tile([128, 1024], F32R, tag="pt")
            for i in range(8):  # c-chunk
                nc.tensor.transpose(
                    pt[:, i * 128 : (i + 1) * 128],
                    ctx_sb[:, i * D + j * 128 : i * D + (j + 1) * 128],
                    self.ident_sb[:],
                )
            dst = cT_sb[:, j * NCTX : (j + 1) * NCTX]
            if j % 2 == 0:
                nc.vector.tensor_copy(dst, pt[:])
            else:
                nc.scalar.copy(dst, pt[:])

    def mm1(self, b):
        nc = self.nc
        qT_sb, cT_sb = self.qT_sb[b], self.cT_sb[b]
        pa = [
            self.pa_pool.tile([128, 512], F32, tag="pa", name=f"pa{b}_{n}")
            for n in range(2)
        ]
        for n in range(2):
            for j in range(8):
                nc.tensor.matmul(
                    pa[n][:],
                    qT_sb[:, j * NQ : (j + 1) * NQ],
                    cT_sb[:, j * NCTX + n * 512 : j * NCTX + (n + 1) * 512],
                    start=(j == 0),
                    stop=(j == 7),
                )
        return pa

    def leaky_sq(self, b, pa, h):
        """Half h: lk = max(0.1*pa, pa); lsq = lk^2; sumsq matmul."""
        nc = self.nc
        sl = slice(h * 512, (h + 1) * 512)
        if h == 0:
            self._lk = self.w1.tile([128, 1024], F32, tag="lk")
            self._lsq = self.w1.tile([128, 1024], F32R, tag="lsq")
        lk, lsq = self._lk, self._lsq
        nc.scalar.activation(lk[:, sl], pa[h][:], AF.Prelu, alpha=NEG_SLOPE)
        nc.vector.tensor_tensor(
            lsq[:, sl], lk[:, sl], lk[:, sl], op=OP.mult
        )
        ps = self.pa_pool.tile([128, 512], F32, tag="pa", name=f"ps2_{b}_{h}")
        nc.tensor.matmul(
            ps[:], self.ones_sb[:], lsq[:, sl], start=True, stop=True
        )
        return lk, ps

    def softmax(self, b, lk, ps2):
        """rsqrt via exp(-0.5*ln(x)) keeps every ACT op in one LUT set
        (natural_log_exp_and_others) -> no LoadActFuncSet switches."""
        nc = self.nc
        lnt = self.w1.tile([128, 1024], F32, tag="lnt")
        ru = lnt
        x9 = self.w1.tile([128, 1024], F32, tag="x9")
        ex = self.w1.tile([128, 1024], F32, tag="ex")
        for h in range(2):
            sl = slice(h * 512, (h + 1) * 512)
            nc.scalar.activation(lnt[:, sl], ps2[h][:], AF.Ln)
            nc.scalar.activation(ru[:, sl], lnt[:, sl], AF.Exp, scale=-0.5)
            nc.vector.scalar_tensor_tensor(
                x9[:, sl], lk[:, sl], SMOOTH, ru[:, sl], op0=OP.mult, op1=OP.mult
            )
        s_tot = self.w1.tile([128, 1], F32, tag="s_tot")
        nc.scalar.activation(ex[:], x9[:], AF.Exp, accum_out=s_tot[:])
        rs = self.w1.tile([128, 1], F32, tag="rs")
        nc.vector.reciprocal(rs[:], s_tot[:])
        attn_sb = self.w1.tile([128, 1024], F32R, tag="attn")
        for h in range(2):
            sl = slice(h * 512, (h + 1) * 512)
            nc.vector.tensor_scalar_mul(attn_sb[:, sl], ex[:, sl], rs[:])
            nc.sync.dma_start(self.attn_ap[b][:, sl], attn_sb[:, sl])
        return attn_sb

    def mm2(self, b, attn_sb):
        nc = self.nc
        et_sb = self.w1.tile([128, 1024], F32R, tag="et")
        pt = self.pt_pool.tile([128, 1024], F32R, tag="pt")
        for i in range(8):
            nc.tensor.transpose(
                pt[:, i * 128 : (i + 1) * 128],
                attn_sb[:, i * 128 : (i + 1) * 128],
                self.ident_sb[:],
            )
        nc.vector.tensor_copy(et_sb[:], pt[:])

        ctx_sb = self.ctx_sb[b]
        pw = [
            self.pw_pool.tile([128, 512], F32, tag="pw", name=f"pw{b}_{n}")
            for n in range(2)
        ]
        w_sb = self.w1.tile([128, 1024], F32, tag="w_sb")
        for n in range(2):
            for i in range(8):
                nc.tensor.matmul(
                    pw[n][:],
                    et_sb[:, i * 128 : (i + 1) * 128],
                    ctx_sb[:, i * D + n * 512 : i * D + (n + 1) * 512],
                    start=(i == 0),
                    stop=(i == 7),
                )
            nc.scalar.copy(w_sb[:, n * 512 : (n + 1) * 512], pw[n][:])
            nc.sync.dma_start(
                self.w_ap[b][:, n * 512 : (n + 1) * 512],
                w_sb[:, n * 512 : (n + 1) * 512],
            )
        # inputs for batch b fully consumed
        del self.ctx_sb[b], self.qT_sb[b], self.cT_sb[b]

    def run(self):
        nb = self.nb
        # prologue: stage batch 0 fully, prefetch batch 1
        self.load(0)
        self.nc.gpsimd.dma_start(self.ones_sb[:], self._ones_d.ap())
        if nb > 1:
            self.load(1)
        self.transpose_ctx(0, range(8))
        for b in range(nb):
            pa = self.mm1(b)
            lk, ps0 = self.leaky_sq(b, pa, 0)
            if b + 2 < nb:
                self.load(b + 2)
            if b + 1 < nb:
                self.transpose_ctx(b + 1, range(0, 2))
            _, ps1 = self.leaky_sq(b, pa, 1)
            if b + 1 < nb:
                self.transpose_ctx(b + 1, range(2, 5))
            attn_sb = self.softmax(b, lk, [ps0, ps1])
            if b + 1 < nb:
                self.transpose_ctx(b + 1, range(5, 8))
            if b == nb - 2:
                # Tail: start the final batch's mm1 + softmax before mm2(b)
                # so its ACT/DVE chain overlaps both mm2s on the PE.
                bl = nb - 1
                pa_l = self.mm1(bl)
                lk_l, ps0_l = self.leaky_sq(bl, pa_l, 0)
                _, ps1_l = self.leaky_sq(bl, pa_l, 1)
                attn_l = self.softmax(bl, lk_l, [ps0_l, ps1_l])
                self.mm2(b, attn_sb)
                self.mm2(bl, attn_l)
                break
            self.mm2(b, attn_sb)


_NC_CACHE = {}


def get_nc(nb: int):
    if nb not in _NC_CACHE:
        _NC_CACHE[nb] = build_nc(nb)
    return _NC_CACHE[nb]


def make_in_maps(query: np.ndarray, context: np.ndarray):
    """Shard full inputs into per-core input maps."""
    n = query.shape[0]
    per = n // N_CORES
    # qT[b, d, q] -> [b, p, j, q] with d = j*128 + p: partition-contiguous DMA
    qT = query.transpose(0, 2, 1).reshape(n, 8, 128, NQ)
    qT = np.ascontiguousarray(qT.transpose(0, 2, 1, 3)).reshape(n, 128, 8 * NQ)
    ident = np.eye(128, dtype=np.float32)
    ones = np.ones((128, 128), dtype=np.float32)
    in_maps = []
    for c in range(N_CORES):
        sl = slice(c * per, (c + 1) * per)
        in_maps.append(
            {
                "queryT": qT[sl],
                "context": np.ascontiguousarray(context[sl]),
                "ident": ident,
                "ones": ones,
            }
        )
    return in_maps


def kernel(query: np.ndarray, context: np.ndarray):
    query = np.asarray(query, dtype=np.float32)
    context = np.asarray(context, dtype=np.float32)
    assert query.shape == (B_TOTAL, NQ, D) and context.shape == (B_TOTAL, NCTX, D)
    per = B_TOTAL // N_CORES

    nc = get_nc(per)
    in_maps = make_in_maps(query, context)
    res = run_bass_kernel_spmd(nc, in_maps, list(range(N_CORES)))
    attn = np.concatenate([r["attn"] for r in res.results], axis=0)
    wctx = np.concatenate([r["wcontext"] for r in res.results], axis=0)
    return (query, wctx, attn)
